# revision 1
# baseline (speedup 1.0000x reference)
"""Trainium2 Bass kernel for nn_CCM_73985106641118 (vq_codebook).

Data-parallel across the batch dim: core b processes batch b (8 cores, B=8).
Within a core, activations are kept feature-major ([feature chunk -> 128
partitions, tokens -> free dim]); each GEMM picks its stationary operand so the
output lands in whichever orientation the consumer needs, which avoids nearly
all transposes.

Precision: the bern_u < P comparison is discontinuous, so the chain that
produces P (x -> H1 -> Hm -> logits) runs in true fp32 matmuls. Everything
downstream of the mask is smooth and runs in fp32r (4x faster on the PE).

DMAs are batched into ~1MB transfers (per-DMA fixed overhead is ~1.3us).
"""

import numpy as np

import concourse.bacc as bacc
import concourse.mybir as mybir
from concourse.masks import make_identity
from concourse.tile import TileContext

f32 = mybir.dt.float32
f32r = mybir.dt.float32r
AX = mybir.AxisListType.X
OP = mybir.AluOpType
AF = mybir.ActivationFunctionType

B, N, C, H, K = 8, 2048, 512, 512, 64
NCP = N // 128   # 16 token chunks of 128
NCJ = N // 512   # 4 token chunks of 512
HC = H // 128    # 4 feature chunks of 128
SCALE = 1.0 / np.sqrt(np.float32(H))

_CACHE = {}


def s128(i):
    return slice(i * 128, (i + 1) * 128)


def s512(i):
    return slice(i * 512, (i + 1) * 512)


def build_nc(debug=False, upto=99):
    nc = bacc.Bacc("TRN2", target_bir_lowering=False, debug=False, num_devices=8)

    x_d = nc.dram_tensor("x", [N, C], f32, kind="ExternalInput").ap()
    bu_d = nc.dram_tensor("bern_u", [N, K], f32, kind="ExternalInput").ap()
    E_d = nc.dram_tensor("cluster_embeddings", [K, H], f32, kind="ExternalInput").ap()
    w1_d = nc.dram_tensor("mlp_w1", [C, H], f32, kind="ExternalInput").ap()
    b1_d = nc.dram_tensor("mlp_b1", [H], f32, kind="ExternalInput").ap()
    w2_d = nc.dram_tensor("mlp_w2", [H, H], f32, kind="ExternalInput").ap()
    b2_d = nc.dram_tensor("mlp_b2", [H], f32, kind="ExternalInput").ap()
    wq_d = nc.dram_tensor("wq", [H, H], f32, kind="ExternalInput").ap()
    wqb_d = nc.dram_tensor("wq_b", [H], f32, kind="ExternalInput").ap()
    wk_d = nc.dram_tensor("wk", [H, H], f32, kind="ExternalInput").ap()
    wkb_d = nc.dram_tensor("wk_b", [H], f32, kind="ExternalInput").ap()
    wout_d = nc.dram_tensor("wout", [H, C], f32, kind="ExternalInput").ap()
    woutb_d = nc.dram_tensor("wout_b", [C], f32, kind="ExternalInput").ap()
    Y_d = nc.dram_tensor("Y", [N, C], f32, kind="ExternalOutput").ap()
    Co_d = nc.dram_tensor("C_out", [N, H], f32, kind="ExternalOutput").ap()
    dbg = {}
    if debug:
        for nm, shp in [("P_dbg", [N, K]), ("M_dbg", [N, K]), ("Hm_dbg", [N, H]),
                        ("Ct_dbg", [64, H]), ("CpT_dbg", [H, N])]:
            dbg[nm] = nc.dram_tensor(nm, shp, f32, kind="ExternalOutput").ap()

    with TileContext(nc) as tc:
        with (
            tc.tile_pool(name="big", bufs=4) as big,
            tc.tile_pool(name="med", bufs=1) as med,
            tc.tile_pool(name="sm", bufs=3) as sm,
            tc.tile_pool(name="psA", bufs=3, space="PSUM") as psA,
            tc.tile_pool(name="psT", bufs=2, space="PSUM") as psT,
            tc.tile_pool(name="psP", bufs=1, space="PSUM") as psP,
            tc.tile_pool(name="psS", bufs=1, space="PSUM") as psS,
        ):
            v = nc.vector
            sc = nc.scalar
            te = nc.tensor

            # staging slots for 1MB batched transfers ([128, 2048] each)
            def stage_tile(name):
                return sm.tile([128, N], f32, tag="xq", bufs=2, name=name)

            # ---- constants / weights -------------------------------------
            if upto >= 1:
                ident = med.tile([128, 128], f32, tag="ident")
                make_identity(nc, ident[:])

                def load_w(dram, tag, dtype):
                    st = stage_tile(f"stage_{tag}")
                    nc.sync.dma_start(
                        out=st[:].rearrange("p (q h) -> p q h", q=4),
                        in_=dram.rearrange("(q p) h -> p q h", p=128))
                    tiles = []
                    for cc in range(HC):
                        t = med.tile([128, 512], dtype, tag="w", bufs=8,
                                     name=f"{tag}{cc}")
                        v.tensor_copy(t[:], st[:, s512(cc)])
                        tiles.append(t)
                    return tiles

                # bern_u in one DMA: [128, 16*64], chunk ncp at cols ncp*64
                bern = med.tile([128, NCP * K], f32, tag="bern")
                nc.sync.dma_start(out=bern[:].rearrange("p (q k) -> p q k", q=16),
                                  in_=bu_d.rearrange("(q p) k -> p q k", p=128))

            # ---- phase 1: xn = l2norm(x) rows, transposed to xnT ---------
            if upto >= 2:
                xnT = [big.tile([128, N], f32, tag="A", name=f"xnT{i}") for i in range(HC)]
                for j in range(NCJ):
                    xq = stage_tile(f"xq{j}")
                    nc.sync.dma_start(
                        out=xq[:].rearrange("p (q c) -> p q c", q=4),
                        in_=x_d[s512(j), :].rearrange("(q p) c -> p q c", p=128))
                    for q in range(4):
                        ncp = j * 4 + q
                        xt = xq[:, s512(q)]
                        xsq = sm.tile([128, C], f32, tag="xsq", bufs=2, name="xsq")
                        sc.activation(xsq[:], xt, AF.Square)
                        ssq = sm.tile([128, 1], f32, tag="ssq", bufs=2, name="ssq")
                        v.reduce_sum(ssq[:], xsq[:], axis=AX)
                        nrm = sm.tile([128, 1], f32, tag="nrm", bufs=2, name="nrm")
                        sc.sqrt(nrm[:], ssq[:])
                        nrm2 = sm.tile([128, 1], f32, tag="nrm2", bufs=2, name="nrm2")
                        v.tensor_scalar(nrm2[:], nrm[:], 1e-12, None, OP.max)
                        inv = sm.tile([128, 1], f32, tag="inv", bufs=2, name="inv")
                        v.reciprocal(inv[:], nrm2[:])
                        xn = sm.tile([128, C], f32, tag="xn", bufs=1, name="xn")
                        v.tensor_scalar(xn[:], xt, inv[:], None, OP.mult)
                        for cc in range(HC):
                            pt = psT.tile([128, 128], f32, tag="pt128")
                            te.transpose(pt[:], xn[:, s128(cc)], ident[:])
                            v.tensor_copy(xnT[cc][:, s128(ncp)], pt[:])

            # ---- weights (loaded after x so staging slots don't deadlock)
            if upto >= 1:
                w1 = load_w(w1_d, "w1", f32)
                w2 = load_w(w2_d, "w2", f32)
                wk_r = load_w(wk_d, "wk", f32r)
                wq_r = load_w(wq_d, "wq", f32r)
                wout_r = load_w(wout_d, "wo", f32r)

                E_f = med.tile([64, H], f32, tag="E")
                nc.sync.dma_start(out=E_f[:], in_=E_d[:, :])
                E_r = med.tile([64, H], f32r, tag="Er")
                v.tensor_copy(E_r[:], E_f[:])

                # per-chunk bias columns [128, 4]
                def bias_cols(dram, tag):
                    t = med.tile([128, HC], f32, tag=tag, name=tag)
                    nc.sync.dma_start(out=t[:], in_=dram.rearrange("(j p) -> p j", p=128))
                    return t

                b1c = bias_cols(b1_d, "b1c")
                wkbc = bias_cols(wkb_d, "wkbc")
                wqbc0 = bias_cols(wqb_d, "wqbc0")
                wqbc = med.tile([128, HC], f32, tag="wqbc")
                v.tensor_scalar(wqbc[:], wqbc0[:], float(SCALE), None, OP.mult)

                b2row = med.tile([1, H], f32, tag="b2row")
                nc.sync.dma_start(out=b2row[:], in_=b2_d.rearrange("(o a) -> o a", o=1))
                wobrow = med.tile([1, C], f32, tag="wobrow")
                nc.sync.dma_start(out=wobrow[:], in_=woutb_d.rearrange("(o a) -> o a", o=1))
                ones128 = med.tile([1, 128], f32, tag="ones")
                nc.gpsimd.memset(ones128[:], 1.0)

                def bcast_row(row, tag):
                    pp = psA.tile([128, 512], f32, tag="psA", name="psA")
                    te.matmul(pp[:], ones128[:], row[:], start=True, stop=True)
                    t = med.tile([128, 512], f32, tag="bcast", bufs=3, name=tag)
                    v.tensor_copy(t[:], pp[:])
                    return t

                b2_bc = bcast_row(b2row, "b2bc")
                wob_bc = bcast_row(wobrow, "wobbc")

                # E row norms -> Ebar (rows scaled to unit norm) and transposes
                esq = med.tile([64, H], f32, tag="esq")
                sc.activation(esq[:], E_f[:], AF.Square)
                ensq = med.tile([64, 1], f32, tag="ensq")
                v.reduce_sum(ensq[:], esq[:], axis=AX)
                enrm = med.tile([64, 1], f32, tag="enrm")
                sc.sqrt(enrm[:], ensq[:])
                einv = med.tile([64, 1], f32, tag="einv")
                v.reciprocal(einv[:], enrm[:])
                Ebar = med.tile([64, H], f32, tag="Ebar")
                v.tensor_scalar(Ebar[:], E_f[:], einv[:], None, OP.mult)

                EbarT, ET = [], []
                for hc in range(HC):
                    pt = psT.tile([128, 64], f32, tag="ptp", bufs=1)
                    te.transpose(pt[:], Ebar[:, s128(hc)], ident[0:64, 0:64])
                    t = med.tile([128, 64], f32, tag=f"ebt{hc}", name=f"ebt{hc}")
                    v.tensor_copy(t[:], pt[:])
                    EbarT.append(t)
                    pt2 = psT.tile([128, 64], f32, tag="ptp", bufs=1)
                    te.transpose(pt2[:], E_f[:, s128(hc)], ident[0:64, 0:64])
                    t2 = med.tile([128, 64], f32r, tag=f"et{hc}", name=f"et{hc}")
                    v.tensor_copy(t2[:], pt2[:])
                    ET.append(t2)


            # ---- phase 2: H1T = relu(w1.T @ xnT + b1)  (fp32) ------------
            if upto >= 3:
                H1T = [big.tile([128, N], f32, tag="B", name=f"H1T{i}") for i in range(HC)]
                for h1c in range(HC):
                    for ncj in range(NCJ):
                        pp = psA.tile([128, 512], f32, tag="psA", name="psA")
                        for cc in range(HC):
                            te.matmul(pp[:], w1[cc][:, s128(h1c)], xnT[cc][:, s512(ncj)],
                                      start=(cc == 0), stop=(cc == HC - 1))
                        sc.activation(H1T[h1c][:, s512(ncj)], pp[:], AF.Relu,
                                      bias=b1c[:, h1c:h1c + 1], scale=1.0)

            # ---- phase 3: Hm token-major = H1 @ w2 + b2  (fp32) ----------
            if upto >= 4:
                # HmB[j][:, q*512:(q+1)*512] holds token chunk ncp=4j+q
                HmB = [big.tile([128, N], f32r, tag="A", name=f"HmB{i}")
                       for i in range(NCJ)]
                for ncp in range(NCP):
                    pp = psA.tile([128, 512], f32, tag="psA", name="psA")
                    for h1c in range(HC):
                        te.matmul(pp[:], H1T[h1c][:, s128(ncp)], w2[h1c][:],
                                  start=(h1c == 0), stop=(h1c == HC - 1))
                    dst = HmB[ncp // 4][:, s512(ncp % 4)]
                    v.tensor_tensor(dst, pp[:], b2_bc[:], OP.add)
                    if debug:
                        nc.sync.dma_start(out=dbg["Hm_dbg"][s128(ncp), :],
                                          in_=dst.bitcast(f32))

                def hm_chunk(ncp, hc):
                    j, q = ncp // 4, ncp % 4
                    return HmB[j][:, q * 512 + hc * 128: q * 512 + (hc + 1) * 128]

            # ---- phase 4: HmT feature-major via PE transposes ------------
            if upto >= 5:
                HmT = [big.tile([128, N], f32r, tag="C", name=f"HmT{i}")
                       for i in range(HC)]
                for ncp in range(NCP):
                    for hc in range(HC):
                        pt = psT.tile([128, 128], f32, tag="pt128")
                        te.transpose(pt[:], hm_chunk(ncp, hc).bitcast(f32), ident[:])
                        v.tensor_copy(HmT[hc][:, s128(ncp)], pt[:])

            # ---- phase 5: logits -> P -> M, and PT / MT ------------------
            if upto >= 6:
                PT = med.tile([64, N], f32r, tag="PT")
                MT = big.tile([64, N], f32r, tag="B", name="MT")
                for ncp in range(NCP):
                    pl = psP.tile([128, 64], f32, tag="psP", name="psP")
                    for hc in range(HC):
                        te.matmul(pl[:], HmT[hc][:, s128(ncp)].bitcast(f32),
                                  EbarT[hc][:], start=(hc == 0), stop=(hc == HC - 1))
                    expP = sm.tile([128, 64], f32, tag="expP", bufs=2, name="expP")
                    se = sm.tile([128, 1], f32, tag="se", bufs=2, name="se")
                    sc.activation(expP[:], pl[:], AF.Exp, accum_out=se[:])
                    invp = sm.tile([128, 1], f32, tag="invp", bufs=2, name="invp")
                    v.reciprocal(invp[:], se[:])
                    P = sm.tile([128, 64], f32, tag="P", bufs=2, name="P")
                    v.tensor_scalar(P[:], expP[:], invp[:], None, OP.mult)
                    M = sm.tile([128, 64], f32, tag="M", bufs=2, name="M")
                    v.tensor_tensor(M[:], P[:], bern[:, ncp * 64:(ncp + 1) * 64],
                                    OP.is_gt)
                    if debug:
                        nc.sync.dma_start(out=dbg["P_dbg"][s128(ncp), :], in_=P[:])
                        nc.sync.dma_start(out=dbg["M_dbg"][s128(ncp), :], in_=M[:])
                    ptp = psT.tile([64, 128], f32, tag="ptp", bufs=1)
                    te.transpose(ptp[:], P[:], ident[:])
                    v.tensor_copy(PT[:, s128(ncp)], ptp[:])
                    mtp = psT.tile([64, 128], f32, tag="ptp", bufs=1)
                    te.transpose(mtp[:], M[:], ident[:])
                    v.tensor_copy(MT[:, s128(ncp)], mtp[:])

            # ---- phase 6: KmatT = wk.T @ HmT + wk_b  (fp32r) -------------
            if upto >= 7:
                KmatT = [big.tile([128, N], f32r, tag="D", name=f"KmatT{i}")
                         for i in range(HC)]
                for hc in range(HC):
                    for ncj in range(NCJ):
                        pp = psA.tile([128, 512], f32, tag="psA", name="psA")
                        for h1c in range(HC):
                            te.matmul(pp[:], wk_r[h1c][:, s128(hc)],
                                      HmT[h1c][:, s512(ncj)],
                                      start=(h1c == 0), stop=(h1c == HC - 1))
                        v.tensor_scalar(KmatT[hc][:, s512(ncj)], pp[:],
                                        wkbc[:, hc:hc + 1], None, OP.add)

            # ---- phase 6b: QT = (wq.T @ ET + wq_b) * scale ---------------
            if upto >= 8:
                QT = []
                for hc in range(HC):
                    pq = psP.tile([128, 64], f32, tag="psP", name="psP")
                    for cc in range(HC):
                        te.matmul(pq[:], wq_r[cc][:, s128(hc)], ET[cc][:],
                                  start=(cc == 0), stop=(cc == HC - 1))
                    t = med.tile([128, 64], f32r, tag=f"qt{hc}", name=f"qt{hc}")
                    sc.activation(t[:], pq[:], AF.Identity,
                                  bias=wqbc[:, hc:hc + 1], scale=float(SCALE))
                    QT.append(t)

            # ---- phase 7: scores -> expS, row sums -----------------------
            if upto >= 9:
                expS = big.tile([64, N], f32, tag="B", name="expS")
                pses = []
                for ncj in range(NCJ):
                    ps_ = psS.tile([64, 512], f32, tag="psS", name="psS")
                    for hc in range(HC):
                        te.matmul(ps_[:], QT[hc][:], KmatT[hc][:, s512(ncj)],
                                  start=(hc == 0), stop=(hc == HC - 1))
                    pse = med.tile([64, 1], f32, tag=f"pse{ncj}", name=f"pse{ncj}")
                    sc.activation(expS[:, s512(ncj)], ps_[:], AF.Exp, accum_out=pse[:])
                    pses.append(pse)
                sA = med.tile([64, 1], f32, tag="sA")
                v.tensor_tensor(sA[:], pses[0][:], pses[1][:], OP.add)
                sA2 = med.tile([64, 1], f32, tag="sA2")
                v.tensor_tensor(sA2[:], pses[2][:], pses[3][:], OP.add)
                sA3 = med.tile([64, 1], f32, tag="sA3")
                v.tensor_tensor(sA3[:], sA[:], sA2[:], OP.add)
                invA = med.tile([64, 1], f32, tag="invA")
                v.reciprocal(invA[:], sA3[:])

                # expST: [128 tok, 64 k] chunks packed into [128, 16*64]
                expST = big.tile([128, NCP * 64], f32r, tag="B", name="expST")
                for ncp in range(NCP):
                    pt = psT.tile([128, 64], f32, tag="ptp", bufs=1)
                    te.transpose(pt[:], expS[0:64, s128(ncp)], ident[0:64, 0:64])
                    v.tensor_copy(expST[:, ncp * 64:(ncp + 1) * 64], pt[:])

            # ---- phase 8: C_temp = A @ Hm  (fp32r) -----------------------
            if upto >= 10:
                pc = psS.tile([64, 512], f32, tag="psS", name="psS")
                for ncp in range(NCP):
                    te.matmul(pc[:], expST[:, ncp * 64:(ncp + 1) * 64],
                              HmB[ncp // 4][:, s512(ncp % 4)],
                              start=(ncp == 0), stop=(ncp == NCP - 1))
                Ctemp = med.tile([64, H], f32r, tag="Ctemp")
                v.tensor_scalar(Ctemp[:], pc[:], invA[:], None, OP.mult)
                if debug:
                    nc.sync.dma_start(out=dbg["Ct_dbg"][0:64, :],
                                      in_=Ctemp[:].bitcast(f32))

            # ---- phase 9: C_pre in both orientations (fp32r) -------------
            if upto >= 11:
                CpreT = [big.tile([128, N], f32r, tag="D", name=f"CpreT{i}")
                         for i in range(HC)]
                for hc in range(HC):
                    for ncj in range(NCJ):
                        pp = psA.tile([128, 512], f32, tag="psA", name="psA")
                        te.matmul(pp[:], Ctemp[:, s128(hc)], MT[:, s512(ncj)],
                                  start=True, stop=True)
                        v.tensor_copy(CpreT[hc][:, s512(ncj)], pp[:])
                        if debug:
                            nc.sync.dma_start(out=dbg["CpT_dbg"][s128(hc), s512(ncj)],
                                              in_=CpreT[hc][:, s512(ncj)].bitcast(f32))
                CpT = [big.tile([128, N], f32, tag="A", name=f"CpT{i}")
                       for i in range(NCJ)]
                for ncp in range(NCP):
                    pp = psA.tile([128, 512], f32, tag="psA", name="psA")
                    te.matmul(pp[:], MT[:, s128(ncp)], Ctemp[:], start=True, stop=True)
                    v.tensor_copy(CpT[ncp // 4][:, s512(ncp % 4)], pp[:])

                # norms over tokens per hidden channel
                invn = []
                for hc in range(HC):
                    parts = []
                    for ncj in range(NCJ):
                        csc = sm.tile([128, 512], f32, tag="xsq", bufs=2, name="csc")
                        sc.activation(csc[:], CpreT[hc][:, s512(ncj)].bitcast(f32),
                                      AF.Square)
                        prt = sm.tile([128, 1], f32, tag="cprt", bufs=4, name="cprt")
                        v.reduce_sum(prt[:], csc[:], axis=AX)
                        parts.append(prt)
                    pa = sm.tile([128, 1], f32, tag="cpa", bufs=2, name="cpa")
                    v.tensor_tensor(pa[:], parts[0][:], parts[1][:], OP.add)
                    pb = sm.tile([128, 1], f32, tag="cpb", bufs=2, name="cpb")
                    v.tensor_tensor(pb[:], parts[2][:], parts[3][:], OP.add)
                    csq = med.tile([128, 1], f32, tag=f"csq{hc}", name=f"csq{hc}")
                    v.tensor_tensor(csq[:], pa[:], pb[:], OP.add)
                    cn = med.tile([128, 1], f32, tag=f"cn{hc}", name=f"cn{hc}")
                    sc.sqrt(cn[:], csq[:])
                    cn2 = med.tile([128, 1], f32, tag=f"cn2{hc}", name=f"cn2{hc}")
                    v.tensor_scalar(cn2[:], cn[:], 1e-12, None, OP.max)
                    iv = med.tile([128, 1], f32, tag=f"ivn{hc}", name=f"ivn{hc}")
                    v.reciprocal(iv[:], cn2[:])
                    invn.append(iv)

                # invn as a row + broadcast tile for the token-major scaling
                invnrow = med.tile([1, H], f32, tag="invnrow")
                for hc in range(HC):
                    pt = psT.tile([1, 128], f32, tag="ptp", bufs=1)
                    te.matmul(pt[:], invn[hc][:, 0:1], ident[:], start=True, stop=True)
                    v.tensor_copy(invnrow[0:1, s128(hc)], pt[:])
                ppn = psA.tile([128, 512], f32, tag="psA", name="psA")
                te.matmul(ppn[:], ones128[:], invnrow[:], start=True, stop=True)
                invn_bc = med.tile([128, H], f32, tag="bcast", bufs=3, name="invnbc")
                v.tensor_copy(invn_bc[:], ppn[:])

                # C output (token-major, scaled), batched 1MB stores
                for j in range(NCJ):
                    cb = stage_tile(f"cb{j}")
                    for q in range(4):
                        v.tensor_tensor(cb[:, s512(q)], CpT[j][:, s512(q)],
                                        invn_bc[:], OP.mult)
                    nc.sync.dma_start(
                        out=Co_d[s512(j), :].rearrange("(q p) c -> p q c", p=128),
                        in_=cb[:].rearrange("p (q c) -> p q c", q=4))

            # ---- phase 10: thetaT = E.T @ PT  (fp32r) --------------------
            if upto >= 12:
                thetaT = [big.tile([128, N], f32, tag="B", name=f"thetaT{i}")
                          for i in range(HC)]
                for hc in range(HC):
                    for ncj in range(NCJ):
                        pp = psA.tile([128, 512], f32, tag="psA", name="psA")
                        te.matmul(pp[:], E_r[:, s128(hc)], PT[:, s512(ncj)],
                                  start=True, stop=True)
                        v.tensor_copy(thetaT[hc][:, s512(ncj)], pp[:])

            # ---- phase 11: GT = (HmT + CpreT*invn) * thetaT, in place ----
            if upto >= 13:
                for hc in range(HC):
                    v.scalar_tensor_tensor(CpreT[hc][:], CpreT[hc][:].bitcast(f32),
                                           invn[hc][:], HmT[hc][:].bitcast(f32),
                                           OP.mult, OP.add)
                    v.tensor_tensor(CpreT[hc][:], CpreT[hc][:].bitcast(f32),
                                    thetaT[hc][:], OP.mult)

            # ---- phase 12: Y = G @ wout + wout_b  (fp32r), batched stores
            if upto >= 14:
                for j in range(NCJ):
                    yb = stage_tile(f"yb{j}")
                    for q in range(4):
                        ncp = j * 4 + q
                        pp = psA.tile([128, 512], f32, tag="psA", name="psA")
                        for hc in range(HC):
                            te.matmul(pp[:], CpreT[hc][:, s128(ncp)], wout_r[hc][:],
                                      start=(hc == 0), stop=(hc == HC - 1))
                        v.tensor_tensor(yb[:, s512(q)], pp[:], wob_bc[:], OP.add)
                    nc.sync.dma_start(
                        out=Y_d[s512(j), :].rearrange("(q p) c -> p q c", p=128),
                        in_=yb[:].rearrange("p (q c) -> p q c", q=4))

    nc.finalize()
    return nc


def _get_nc():
    if "nc" not in _CACHE:
        _CACHE["nc"] = build_nc()
    return _CACHE["nc"]


def kernel(**inputs):
    from concourse.bass_utils import run_bass_kernel_spmd

    nc = _get_nc()
    arr = {k: np.ascontiguousarray(np.asarray(v, dtype=np.float32))
           for k, v in inputs.items()}
    shared = {k: arr[k] for k in
              ("cluster_embeddings", "mlp_w1", "mlp_b1", "mlp_w2", "mlp_b2",
               "wq", "wq_b", "wk", "wk_b", "wout", "wout_b")}
    in_maps = [dict(x=arr["x"][b], bern_u=arr["bern_u"][b], **shared)
               for b in range(B)]
    res = run_bass_kernel_spmd(nc, in_maps, list(range(B))).results
    Y = np.stack([res[b]["Y"] for b in range(B)])
    Co = np.stack([res[b]["C_out"] for b in range(B)])
    return (Y, Co)


if __name__ == "__main__":
    import os
    os.environ.setdefault("JAX_PLATFORMS", "cpu")
    from concourse.timeline_sim import TimelineSim

    nc = build_nc()
    ts = TimelineSim(nc, trace=False)
    print("TimelineSim:", ts.simulate(), "ns")



# revision 20
# speedup vs baseline: 1.5768x; 1.5768x over previous
"""Trainium2 Bass kernel for nn_CCM_73985106641118 (vq_codebook).

Data-parallel across the batch dim: core b processes batch b (8 cores, B=8).

v2 design notes (vs the fp32 baseline):
- Every GEMM runs in fp32r (1 cyc/row at N>=512 on the PE vs 4 for fp32);
  empirically fp32r keeps ~1e-5 relative error, far inside the 2e-2 gate.
- Weights are DMA'd straight into single [128, 4*512] fp32r tiles (no
  staging copies through the vector engine).
- Hm is produced feature-major first (w2 chunks stationary) so the b2 bias
  lands in a scalar-engine drain; the token-major copy comes from PE
  transposes batched 4-at-a-time into one PSUM bank per drain.
- The C_pre l2-norms are computed via the Gram matrix  n2[h] =
  Ctemp[:,h]^T (M^T M) Ctemp[:,h]  so C_pre never needs to be
  materialized unscaled; Ctemp is pre-scaled once ([64,512]) and both the
  feature-major (H update) and token-major (C output) products come out
  normalized directly from the PE.
- PSUM drains are fused with the adjacent elementwise op wherever
  possible (relu+bias, bias add, +HmT, *theta, *invA) and spread across
  the scalar and vector engines.
"""

import numpy as np

import concourse.bacc as bacc
import concourse.mybir as mybir
from concourse.masks import make_identity
from concourse.tile import TileContext

f32 = mybir.dt.float32
f32r = mybir.dt.float32r
AX = mybir.AxisListType.X
OP = mybir.AluOpType
AF = mybir.ActivationFunctionType

B, N, C, H, K = 8, 2048, 512, 512, 64
NCP = N // 128   # 16 token chunks of 128
NCJ = N // 512   # 4 token chunks of 512
HC = H // 128    # 4 feature chunks of 128
SCALE = 1.0 / np.sqrt(np.float32(H))

_CACHE = {}


def s128(i):
    return slice(i * 128, (i + 1) * 128)


def s512(i):
    return slice(i * 512, (i + 1) * 512)


def s64(i):
    return slice(i * 64, (i + 1) * 64)


def build_nc(debug=False, upto=99):
    nc = bacc.Bacc("TRN2", target_bir_lowering=False, debug=False, num_devices=8)

    x_d = nc.dram_tensor("x", [N, C], f32, kind="ExternalInput").ap()
    bu_d = nc.dram_tensor("bern_u", [N, K], f32, kind="ExternalInput").ap()
    E_d = nc.dram_tensor("cluster_embeddings", [K, H], f32, kind="ExternalInput").ap()
    w1_d = nc.dram_tensor("mlp_w1", [C, H], f32, kind="ExternalInput").ap()
    b1_d = nc.dram_tensor("mlp_b1", [H], f32, kind="ExternalInput").ap()
    w2_d = nc.dram_tensor("mlp_w2", [H, H], f32, kind="ExternalInput").ap()
    b2_d = nc.dram_tensor("mlp_b2", [H], f32, kind="ExternalInput").ap()
    wq_d = nc.dram_tensor("wq", [H, H], f32, kind="ExternalInput").ap()
    wqb_d = nc.dram_tensor("wq_b", [H], f32, kind="ExternalInput").ap()
    wk_d = nc.dram_tensor("wk", [H, H], f32, kind="ExternalInput").ap()
    wkb_d = nc.dram_tensor("wk_b", [H], f32, kind="ExternalInput").ap()
    wout_d = nc.dram_tensor("wout", [H, C], f32, kind="ExternalInput").ap()
    woutb_d = nc.dram_tensor("wout_b", [C], f32, kind="ExternalInput").ap()
    Y_d = nc.dram_tensor("Y", [N, C], f32, kind="ExternalOutput").ap()
    Co_d = nc.dram_tensor("C_out", [N, H], f32, kind="ExternalOutput").ap()
    dbg = {}
    if debug:
        for nm, shp in [("P_dbg", [N, K]), ("M_dbg", [N, K]), ("HmT_dbg", [H, N]),
                        ("Ct_dbg", [64, H]), ("n2_dbg", [1, H])]:
            dbg[nm] = nc.dram_tensor(nm, shp, f32, kind="ExternalOutput").ap()

    with TileContext(nc) as tc:
        with (
            tc.tile_pool(name="big", bufs=4) as big,
            tc.tile_pool(name="med", bufs=1) as med,
            tc.tile_pool(name="sm", bufs=2) as sm,
            tc.tile_pool(name="stg", bufs=2) as stg,
            tc.tile_pool(name="psA", bufs=3, space="PSUM") as psA,
            tc.tile_pool(name="psT", bufs=2, space="PSUM") as psT,
            tc.tile_pool(name="psP", bufs=2, space="PSUM") as psP,
            tc.tile_pool(name="psS", bufs=1, space="PSUM") as psS,
        ):
            v = nc.vector
            sc = nc.scalar
            te = nc.tensor
            gp = nc.gpsimd

            # ---- constants / weights (no x dependency) -------------------
            ident = med.tile([128, 128], f32, tag="ident")
            make_identity(nc, ident[:])

            def load_w(dram, tag):
                # 5 weights rotate through 3 slots: w1/w2 are dead after
                # phases 2/3, so wq/wout copies simply wait for those readers
                # (those two are loaded after the phase-2 code below so the
                # gpsimd queue order matches the dependency order).
                # fp32r matmul operands must be rounded by an engine op, so
                # stage each quarter and round via the (otherwise idle)
                # gpsimd engine.
                t = med.tile([128, HC * 512], f32r, tag="w", bufs=3, name=tag)
                for cc in range(HC):
                    st = sm.tile([128, 512], f32, tag="wst", bufs=2,
                                 name=f"wst_{tag}{cc}")
                    nc.sync.dma_start(out=st[:], in_=dram[s128(cc), :])
                    gp.tensor_copy(t[:, s512(cc)], st[:])
                return t

            w1t = load_w(w1_d, "w1t")
            w2t = load_w(w2_d, "w2t")
            wkt = load_w(wk_d, "wkt")

            # chunk (cc) stationary slice: rows cc*128.. of the [512, 512]
            # weight, columns block*128..
            def wchunk(t, cc, blk):
                return t[:, cc * 512 + blk * 128: cc * 512 + (blk + 1) * 128]

            E_f = med.tile([64, H], f32, tag="E")
            nc.sync.dma_start(out=E_f[:], in_=E_d[:, :])
            E_r = med.tile([64, H], f32r, tag="Er")
            gp.tensor_copy(E_r[:], E_f[:])

            def bias_cols(dram, tag):
                t = med.tile([128, HC], f32, tag=tag, name=tag)
                nc.sync.dma_start(out=t[:], in_=dram.rearrange("(j p) -> p j", p=128))
                return t

            b1c = bias_cols(b1_d, "b1c")
            b2c = bias_cols(b2_d, "b2c")
            wkbc = bias_cols(wkb_d, "wkbc")
            wqbc0 = bias_cols(wqb_d, "wqbc0")
            wqbc = med.tile([128, HC], f32, tag="wqbc")
            v.tensor_scalar(wqbc[:], wqbc0[:], float(SCALE), None, OP.mult)

            wobrow = med.tile([1, C], f32, tag="wobrow")
            nc.sync.dma_start(out=wobrow[:], in_=woutb_d.rearrange("(o a) -> o a", o=1))
            ones128 = med.tile([1, 128], f32, tag="ones")
            gp.memset(ones128[:], 1.0)
            ones64c = med.tile([64, 1], f32, tag="ones64c")
            gp.memset(ones64c[:], 1.0)
            ones1_64 = med.tile([1, 64], f32, tag="ones1_64")
            gp.memset(ones1_64[:], 1.0)

            ppn = psA.tile([128, 512], f32, tag="mm", name="ppn")
            te.matmul(ppn[:], ones128[:], wobrow[:], start=True, stop=True)
            wob_bc = med.tile([128, C], f32, tag="wob_bc")
            v.tensor_copy(wob_bc[:], ppn[:])

            # E row norms -> Ebar (unit rows), EbarT / ET feature-major
            esq = sm.tile([128, C], f32, tag="xsq", bufs=1, name="esq")[0:64, :]
            ensq = med.tile([64, 1], f32, tag="ensq")
            sc.activation(esq[:], E_f[:], AF.Square, accum_out=ensq[:])
            enrm = med.tile([64, 1], f32, tag="enrm")
            sc.sqrt(enrm[:], ensq[:])
            einv = med.tile([64, 1], f32, tag="einv")
            v.reciprocal(einv[:], enrm[:])
            Ebar = med.tile([64, H], f32, tag="Ebar")
            v.tensor_scalar(Ebar[:], E_f[:], einv[:], None, OP.mult)

            EbarT, ET = [], []
            for hc in range(HC):
                pt = psP.tile([128, 64], f32, tag="small", bufs=2, name="ptE")
                te.transpose(pt[:], Ebar[:, s128(hc)], ident[0:64, 0:64])
                t = med.tile([128, 64], f32r, tag=f"ebt{hc}", name=f"ebt{hc}")
                v.tensor_copy(t[:], pt[:])
                EbarT.append(t)
                pt2 = psP.tile([128, 64], f32, tag="small", bufs=2, name="ptE2")
                te.transpose(pt2[:], E_f[:, s128(hc)], ident[0:64, 0:64])
                t2 = med.tile([128, 64], f32r, tag=f"et{hc}", name=f"et{hc}")
                v.tensor_copy(t2[:], pt2[:])
                ET.append(t2)

            # bern in one DMA: [128, 16*64], chunk ncp at cols ncp*64
            bern = med.tile([128, NCP * K], f32, tag="bern")
            nc.sync.dma_start(out=bern[:].rearrange("p (q k) -> p q k", q=16),
                              in_=bu_d.rearrange("(q p) k -> p q k", p=128))

            # ---- phase 1: xn = l2norm(x) rows (in place), xnT fp32r ------
            if upto >= 1:
                xnT = [big.tile([128, N], f32r, tag="A", name=f"xnT{i}")
                       for i in range(HC)]
                for j in range(NCJ):
                    xq = stg.tile([128, N], f32, tag="xq", bufs=2, name=f"xq{j}")
                    nc.sync.dma_start(
                        out=xq[:].rearrange("p (q c) -> p q c", q=4),
                        in_=x_d[s512(j), :].rearrange("(q p) c -> p q c", p=128))
                    ssq = sm.tile([128, 4], f32, tag="ssq", bufs=2, name="ssq")
                    xsq = sm.tile([128, C], f32, tag="xsq", bufs=1, name="xsq")
                    for q in range(4):
                        sc.activation(xsq[:], xq[:, s512(q)], AF.Square,
                                      accum_out=ssq[:, q:q + 1])
                    nrm = sm.tile([128, 4], f32, tag="nrm", bufs=2, name="nrm")
                    sc.sqrt(nrm[:], ssq[:])
                    nrm2 = sm.tile([128, 4], f32, tag="nrm2", bufs=2, name="nrm2")
                    v.tensor_scalar(nrm2[:], nrm[:], 1e-12, None, OP.max)
                    inv = sm.tile([128, 4], f32, tag="inv", bufs=2, name="inv")
                    v.reciprocal(inv[:], nrm2[:])
                    for q in range(4):
                        v.tensor_scalar(xq[:, s512(q)], xq[:, s512(q)],
                                        inv[:, q:q + 1], None, OP.mult)
                    for cc in range(HC):
                        pt = psT.tile([128, 512], f32, tag="tr", name="ptx")
                        for q in range(4):
                            te.transpose(pt[:, s128(q)],
                                         xq[:, q * 512 + cc * 128:
                                            q * 512 + (cc + 1) * 128],
                                         ident[:])
                        v.tensor_copy(xnT[cc][:, s512(j)], pt[:])

            # ---- phase 2: H1T = relu(w1.T @ xnT + b1)  fp32r -------------
            if upto >= 2:
                H1T = [big.tile([128, N], f32r, tag="B", name=f"H1T{i}")
                       for i in range(HC)]
                for j in range(NCJ):
                    for h1c in range(HC):
                        pp = psA.tile([128, 512], f32, tag="mm", name="ppH1")
                        for cc in range(HC):
                            te.matmul(pp[:], wchunk(w1t, cc, h1c),
                                      xnT[cc][:, s512(j)],
                                      start=(cc == 0), stop=(cc == HC - 1))
                        sc.activation(H1T[h1c][:, s512(j)], pp[:], AF.Relu,
                                      bias=b1c[:, h1c:h1c + 1], scale=1.0)

            # wq/wout reuse the w1/w2 slots; loading them here (after the
            # phase-2 trace) keeps the gpsimd queue deadlock-free.
            wqt = load_w(wq_d, "wqt")
            wot = load_w(wout_d, "wot")

            # ---- QT = (wq.T @ ET + wq_b) * scale -------------------------
            QT = []
            for hc in range(HC):
                pq = psP.tile([128, 64], f32, tag="small", bufs=2, name="pq")
                for cc in range(HC):
                    te.matmul(pq[:], wchunk(wqt, cc, hc), ET[cc][:],
                              start=(cc == 0), stop=(cc == HC - 1))
                t = med.tile([128, 64], f32r, tag=f"qt{hc}", name=f"qt{hc}")
                sc.activation(t[:], pq[:], AF.Identity,
                              bias=wqbc[:, hc:hc + 1], scale=float(SCALE))
                QT.append(t)

            # ---- phase 3: HmT = w2.T @ H1T + b2 (feature-major) ----------
            if upto >= 3:
                HmT = [big.tile([128, N], f32r, tag="C", name=f"HmT{i}")
                       for i in range(HC)]
                for j in range(NCJ):
                    for hc in range(HC):
                        pp = psA.tile([128, 512], f32, tag="mm", name="ppHm")
                        for h1c in range(HC):
                            te.matmul(pp[:], wchunk(w2t, h1c, hc),
                                      H1T[h1c][:, s512(j)],
                                      start=(h1c == 0), stop=(h1c == HC - 1))
                        sc.activation(HmT[hc][:, s512(j)], pp[:], AF.Identity,
                                      bias=b2c[:, hc:hc + 1], scale=1.0)
                        if debug:
                            nc.sync.dma_start(
                                out=dbg["HmT_dbg"][s128(hc), s512(j)],
                                in_=HmT[hc][:, s512(j)].bitcast(f32))

            # ---- phase 4: HmB token-major via PE transposes --------------
            if upto >= 4:
                HmB = [big.tile([128, N], f32r, tag="A", name=f"HmB{i}")
                       for i in range(NCJ)]
                for j in range(NCJ):
                    for q in range(4):
                        ncp = 4 * j + q
                        pt = psT.tile([128, 512], f32, tag="tr", name="ptm")
                        for hc in range(HC):
                            te.transpose(pt[:, s128(hc)],
                                         HmT[hc][:, s128(ncp)].bitcast(f32),
                                         ident[:])
                        v.tensor_copy(HmB[j][:, s512(q)], pt[:])

            # ---- phase 5: logits -> P -> M; PT / MT fp32r ----------------
            if upto >= 5:
                PT = med.tile([64, N], f32r, tag="PT")
                MT = med.tile([64, N], f32r, tag="MT")
                Mfull = med.tile([128, NCP * K], f32, tag="Mfull")
                for j in range(NCJ):
                    Pq = []
                    for q in range(4):
                        ncp = 4 * j + q
                        pl = psP.tile([128, 64], f32, tag="small", bufs=2,
                                      name="pl")
                        for hc in range(HC):
                            te.matmul(pl[:], HmT[hc][:, s128(ncp)],
                                      EbarT[hc][:],
                                      start=(hc == 0), stop=(hc == HC - 1))
                        expP = sm.tile([128, 64], f32, tag="expP", bufs=2,
                                       name="expP")
                        se = sm.tile([128, 1], f32, tag="se", bufs=2, name="se")
                        sc.activation(expP[:], pl[:], AF.Exp, accum_out=se[:])
                        invp = sm.tile([128, 1], f32, tag="invp", bufs=2,
                                       name="invp")
                        v.reciprocal(invp[:], se[:])
                        P = sm.tile([128, 64], f32, tag="P", bufs=4, name="P")
                        v.tensor_scalar(P[:], expP[:], invp[:], None, OP.mult)
                        v.tensor_tensor(Mfull[:, s64(ncp)], P[:],
                                        bern[:, s64(ncp)], OP.is_gt)
                        if debug:
                            nc.sync.dma_start(out=dbg["P_dbg"][s128(ncp), :],
                                              in_=P[:])
                            nc.sync.dma_start(out=dbg["M_dbg"][s128(ncp), :],
                                              in_=Mfull[:, s64(ncp)])
                        Pq.append(P)
                    ptp = psT.tile([64, 512], f32, tag="tr", name="ptp")
                    for q in range(4):
                        te.transpose(ptp[:, s128(q)], Pq[q][:], ident[:])
                    v.tensor_copy(PT[:, s512(j)], ptp[:])
                    mtp = psT.tile([64, 512], f32, tag="tr", name="mtp")
                    for q in range(4):
                        te.transpose(mtp[:, s128(q)], Mfull[:, s64(4 * j + q)],
                                     ident[:])
                    v.tensor_copy(MT[:, s512(j)], mtp[:])

                # Gram matrix Gm = M.T @ M (fp32, N=64 outputs)
                gm_ps = psP.tile([64, 64], f32, tag="small", bufs=2, name="gm")
                for ncp in range(NCP):
                    te.matmul(gm_ps[:], Mfull[:, s64(ncp)], Mfull[:, s64(ncp)],
                              start=(ncp == 0), stop=(ncp == NCP - 1))
                Gm = med.tile([64, 64], f32, tag="Gm")
                v.tensor_copy(Gm[:], gm_ps[:])

            # ---- phase 6: KmatT = wk.T @ HmT + wk_b  fp32r ---------------
            if upto >= 6:
                KmatT = [big.tile([128, N], f32r, tag="B", name=f"KmatT{i}")
                         for i in range(HC)]
                for hc in range(HC):
                    for j in range(NCJ):
                        pp = psA.tile([128, 512], f32, tag="mm", name="ppK")
                        for h1c in range(HC):
                            te.matmul(pp[:], wchunk(wkt, h1c, hc),
                                      HmT[h1c][:, s512(j)],
                                      start=(h1c == 0), stop=(h1c == HC - 1))
                        sc.activation(KmatT[hc][:, s512(j)], pp[:], AF.Identity,
                                      bias=wkbc[:, hc:hc + 1], scale=1.0)

            # ---- phase 7: scores -> expS (unnormalized), expST -----------
            if upto >= 7:
                expS = med.tile([64, N], f32, tag="expS")
                pses = []
                for j in range(NCJ):
                    ps_ = psS.tile([64, 512], f32, tag="s64", name="psc")
                    for hc in range(HC):
                        te.matmul(ps_[:], QT[hc][:], KmatT[hc][:, s512(j)],
                                  start=(hc == 0), stop=(hc == HC - 1))
                    pse = med.tile([64, 1], f32, tag=f"pse{j}", name=f"pse{j}")
                    sc.activation(expS[:, s512(j)], ps_[:], AF.Exp,
                                  accum_out=pse[:])
                    pses.append(pse)
                sA = med.tile([64, 1], f32, tag="sA")
                v.tensor_tensor(sA[:], pses[0][:], pses[1][:], OP.add)
                sA2 = med.tile([64, 1], f32, tag="sA2")
                v.tensor_tensor(sA2[:], pses[2][:], pses[3][:], OP.add)
                sA3 = med.tile([64, 1], f32, tag="sA3")
                v.tensor_tensor(sA3[:], sA[:], sA2[:], OP.add)
                invA = med.tile([64, 1], f32, tag="invA")
                v.reciprocal(invA[:], sA3[:])

                expST = med.tile([128, NCP * 64], f32r, tag="expST")
                for j in range(NCJ):
                    pt = psT.tile([128, 256], f32, tag="tr", name="pte")
                    for q in range(4):
                        te.transpose(pt[:, s64(q)],
                                     expS[0:64, s128(4 * j + q)],
                                     ident[0:64, 0:64])
                    v.tensor_copy(expST[:, j * 256:(j + 1) * 256], pt[:])

            # ---- phase 8: Ctemp = (A @ Hm) fp32r -------------------------
            if upto >= 8:
                pc = psS.tile([64, 512], f32, tag="s64", name="pc")
                for ncp in range(NCP):
                    te.matmul(pc[:], expST[:, s64(ncp)],
                              HmB[ncp // 4][:, s512(ncp % 4)],
                              start=(ncp == 0), stop=(ncp == NCP - 1))
                Ctemp = med.tile([64, H], f32, tag="Ctemp")
                v.tensor_scalar(Ctemp[:], pc[:], invA[:], None, OP.mult)
                if debug:
                    nc.sync.dma_start(out=dbg["Ct_dbg"][0:64, :],
                                      in_=Ctemp[:])

            # ---- phase 9: norms via Gram; Ctemp_s; Hupd; C output --------
            if upto >= 9:
                gc_ps = psS.tile([64, 512], f32, tag="s64", name="gc")
                te.matmul(gc_ps[:], Gm[:], Ctemp[:], start=True, stop=True)
                prod = med.tile([64, H], f32, tag="prod")
                v.tensor_tensor(prod[:], Ctemp[:], gc_ps[:], OP.mult)
                n2_ps = psS.tile([1, 512], f32, tag="s64", name="n2")
                te.matmul(n2_ps[:], ones64c[:], prod[:], start=True, stop=True)
                if debug:
                    nroot_d = med.tile([1, H], f32, tag="nroot_d")
                    v.tensor_copy(nroot_d[:], n2_ps[:])
                    nc.sync.dma_start(out=dbg["n2_dbg"][:, :], in_=nroot_d[:])
                nroot = med.tile([1, H], f32, tag="nroot")
                sc.sqrt(nroot[:], n2_ps[:])
                v.tensor_scalar(nroot[:], nroot[:], 1e-12, None, OP.max)
                invn_row = med.tile([1, H], f32, tag="invn_row")
                v.reciprocal(invn_row[:], nroot[:])
                bc_ps = psS.tile([64, 512], f32, tag="s64", name="bc")
                te.matmul(bc_ps[:], ones1_64[:], invn_row[:], start=True,
                          stop=True)
                Ctemp_s = med.tile([64, H], f32r, tag="Ctemp_s")
                v.tensor_tensor(Ctemp_s[:], Ctemp[:], bc_ps[:], OP.mult)

                # H_upd (in place on HmT): HmT += Ctemp_s.T @ MT
                for hc in range(HC):
                    for j in range(NCJ):
                        pp = psA.tile([128, 512], f32, tag="mm", name="ppCp")
                        te.matmul(pp[:], Ctemp_s[:, s128(hc)], MT[:, s512(j)],
                                  start=True, stop=True)
                        v.tensor_tensor(HmT[hc][:, s512(j)], pp[:],
                                        HmT[hc][:, s512(j)], OP.add)

                # C output (already normalized): MT.T @ Ctemp_s, batched DMA
                for j in range(NCJ):
                    cb = stg.tile([128, N], f32, tag="xq", bufs=2, name=f"cb{j}")
                    for q in range(4):
                        pp = psA.tile([128, 512], f32, tag="mm", name="ppCo")
                        te.matmul(pp[:], MT[:, s128(4 * j + q)], Ctemp_s[:],
                                  start=True, stop=True)
                        sc.activation(cb[:, s512(q)], pp[:], AF.Copy)
                    nc.sync.dma_start(
                        out=Co_d[s512(j), :].rearrange("(q p) c -> p q c", p=128),
                        in_=cb[:].rearrange("p (q c) -> p q c", q=4))

            # ---- phase 10/11: G = H_upd * (E.T @ PT), in place -----------
            if upto >= 10:
                for hc in range(HC):
                    for j in range(NCJ):
                        pth = psA.tile([128, 512], f32, tag="mm", name="ppTh")
                        te.matmul(pth[:], E_r[:, s128(hc)], PT[:, s512(j)],
                                  start=True, stop=True)
                        v.tensor_tensor(HmT[hc][:, s512(j)], pth[:],
                                        HmT[hc][:, s512(j)], OP.mult)

            # ---- phase 12: Y = G @ wout + wout_b, batched DMA ------------
            if upto >= 11:
                for j in range(NCJ):
                    yb = stg.tile([128, N], f32, tag="xq", bufs=2, name=f"yb{j}")
                    for q in range(4):
                        ncp = 4 * j + q
                        pp = psA.tile([128, 512], f32, tag="mm", name="ppY")
                        for hc in range(HC):
                            te.matmul(pp[:], HmT[hc][:, s128(ncp)],
                                      wot[:, s512(hc)],
                                      start=(hc == 0), stop=(hc == HC - 1))
                        v.tensor_tensor(yb[:, s512(q)], pp[:], wob_bc[:],
                                        OP.add)
                    nc.sync.dma_start(
                        out=Y_d[s512(j), :].rearrange("(q p) c -> p q c", p=128),
                        in_=yb[:].rearrange("p (q c) -> p q c", q=4))

    nc.finalize()
    return nc


def _get_nc():
    if "nc" not in _CACHE:
        _CACHE["nc"] = build_nc()
    return _CACHE["nc"]


def kernel(**inputs):
    from concourse.bass_utils import run_bass_kernel_spmd

    nc = _get_nc()
    arr = {k: np.ascontiguousarray(np.asarray(v, dtype=np.float32))
           for k, v in inputs.items()}
    shared = {k: arr[k] for k in
              ("cluster_embeddings", "mlp_w1", "mlp_b1", "mlp_w2", "mlp_b2",
               "wq", "wq_b", "wk", "wk_b", "wout", "wout_b")}
    in_maps = [dict(x=arr["x"][b], bern_u=arr["bern_u"][b], **shared)
               for b in range(B)]
    res = run_bass_kernel_spmd(nc, in_maps, list(range(B))).results
    Y = np.stack([res[b]["Y"] for b in range(B)])
    Co = np.stack([res[b]["C_out"] for b in range(B)])
    return (Y, Co)


if __name__ == "__main__":
    import os
    os.environ.setdefault("JAX_PLATFORMS", "cpu")
    from concourse.timeline_sim import TimelineSim

    nc = build_nc()
    ts = TimelineSim(nc, trace=False)
    print("TimelineSim:", ts.simulate(), "ns")
